# revision 2
# baseline (speedup 1.0000x reference)
"""DGCNN (2x EdgeConv kNN=5 + MLP head) Trainium2 kernel, data-parallel over 8 NeuronCores.

Contract: kernel(**inputs) takes the FULL inputs of nn_DEC_41180146434796
(pos [32,2048,3] + MLP weights) and returns the FULL [32,2] output.
Each core processes 4 graphs end-to-end (kNN, gathers, max-aggregations local).
"""
import numpy as np

import concourse.bass as bass
import concourse.mybir as mybir
from concourse import bacc, tile
from concourse import bass_utils
from concourse.masks import make_identity

F32 = mybir.dt.float32
F32R = mybir.dt.float32r
U32 = mybir.dt.uint32
I16 = mybir.dt.int16
AF = mybir.ActivationFunctionType
ALU = mybir.AluOpType
AX = mybir.AxisListType

N = 2048          # nodes per graph
NG = 4            # graphs per core
K = 5             # kNN neighbors (incl self)
NT = 16           # node tiles of 128
NCORES = 8

_CACHE = {}


def _sigma_read(ap):
    """View a [C, 2048] natural-ordered tensor so its free stream is sigma-ordered.

    sigma col s = 16*q + b  <->  node i = 128*b + q.  Iteration (q outer, b inner),
    address = 128*b + q.
    """
    return ap.rearrange("c (b q) -> c q b", b=16, q=128)


def build_nc():
    nc = bacc.Bacc(None, target_bir_lowering=False)

    # ---------------- I/O ----------------
    posT_d = nc.dram_tensor("posT", [NG, 3, N], F32, kind="ExternalInput")
    # folded weights (see kernel() for host-side folding)
    w1a_A_d = nc.dram_tensor("w1aA", [3, 64], F32, kind="ExternalInput")
    w1a_B_d = nc.dram_tensor("w1aB", [3, 64], F32, kind="ExternalInput")
    w1b_d = nc.dram_tensor("w1b", [64, 64], F32, kind="ExternalInput")
    w1c_d = nc.dram_tensor("w1c", [64, 64], F32, kind="ExternalInput")
    w2A_d = nc.dram_tensor("w2A", [64, 128], F32, kind="ExternalInput")
    w2B_d = nc.dram_tensor("w2B", [64, 128], F32, kind="ExternalInput")
    wl1_d = nc.dram_tensor("wl1", [64, 1024], F32, kind="ExternalInput")
    wl2_d = nc.dram_tensor("wl2", [128, 1024], F32, kind="ExternalInput")
    wm1_d = nc.dram_tensor("wm1", [128, 8, 512], F32, kind="ExternalInput")
    wm2_d = nc.dram_tensor("wm2", [128, 4, 256], F32, kind="ExternalInput")
    wout_d = nc.dram_tensor("wout", [128, 2, 2], F32, kind="ExternalInput")
    # biases / scales, per-partition layouts
    b1a_d = nc.dram_tensor("b1a", [64, 1], F32, kind="ExternalInput")
    b1b_d = nc.dram_tensor("b1b", [64, 1], F32, kind="ExternalInput")
    b1c_d = nc.dram_tensor("b1c", [64, 1], F32, kind="ExternalInput")
    s1c_d = nc.dram_tensor("s1c", [64, 1], F32, kind="ExternalInput")
    h1c_d = nc.dram_tensor("h1c", [64, 1], F32, kind="ExternalInput")
    b2_d = nc.dram_tensor("b2", [128, 1], F32, kind="ExternalInput")
    bl_d = nc.dram_tensor("bl", [128, 8], F32, kind="ExternalInput")
    bm1_d = nc.dram_tensor("bm1", [128, 4], F32, kind="ExternalInput")
    bm2_d = nc.dram_tensor("bm2", [128, 2], F32, kind="ExternalInput")
    bout_d = nc.dram_tensor("bout", [2, 1], F32, kind="ExternalInput")

    out_d = nc.dram_tensor("out", [2, NG], F32, kind="ExternalOutput")

    with tile.TileContext(nc) as tc:
        with tc.tile_pool(name="wpool", bufs=1) as wp, \
             tc.tile_pool(name="persist", bufs=1) as pp, \
             tc.tile_pool(name="work", bufs=1) as work, \
             tc.tile_pool(name="workB", bufs=2) as workB, \
             tc.tile_pool(name="ps", bufs=2, space="PSUM") as psp:

            # ---------------- weights to SBUF (one-time) ----------------
            def wload(dram, shape, dtype=F32R, name=None):
                t = wp.tile(shape, dtype, name=name or dram.name + "_s")
                if dtype == F32R:
                    fs = 1
                    for d in shape[1:]:
                        fs *= d
                    if len(shape) == 3:
                        tf = t[:].rearrange("p a b -> p (a b)")
                        df = dram[:].rearrange("p a b -> p (a b)")
                    else:
                        tf, df = t[:], dram[:]
                    for o in range(0, fs, 512):
                        w_ = min(512, fs - o)
                        stg = work.tile([shape[0], w_], F32, tag="wstg")
                        nc.sync.dma_start(stg[:], df[:, o:o + w_])
                        nc.scalar.activation(tf[:, o:o + w_], stg[:], AF.Copy)
                else:
                    nc.sync.dma_start(t[:], dram[:])
                return t

            w1aA = wload(w1a_A_d, [3, 64])
            w1aB = wload(w1a_B_d, [3, 64])
            w1b = wload(w1b_d, [64, 64])
            w1c = wload(w1c_d, [64, 64])
            w2A = wload(w2A_d, [64, 128])
            w2B = wload(w2B_d, [64, 128])
            wl1 = wload(wl1_d, [64, 1024])
            wl2 = wload(wl2_d, [128, 1024])
            wm1 = wload(wm1_d, [128, 8, 512])
            wm2 = wload(wm2_d, [128, 4, 256])
            wout = wload(wout_d, [128, 2, 2])
            b1a = wload(b1a_d, [64, 1], F32)
            b1b = wload(b1b_d, [64, 1], F32)
            b1c = wload(b1c_d, [64, 1], F32)
            s1c = wload(s1c_d, [64, 1], F32)
            h1c = wload(h1c_d, [64, 1], F32)
            b2 = wload(b2_d, [128, 1], F32)
            bl = wload(bl_d, [128, 8], F32)
            bm1 = wload(bm1_d, [128, 4], F32)
            bm2 = wload(bm2_d, [128, 2], F32)
            bout = wload(bout_d, [2, 1], F32)

            ident = wp.tile([128, 128], F32)
            make_identity(nc, ident[:])
            ones3 = wp.tile([3, 1], F32)
            nc.vector.memset(ones3[:], 1.0)
            ones64 = wp.tile([64, 1], F32)
            nc.vector.memset(ones64[:], 1.0)
            onesrow = wp.tile([1, N], F32R)
            nc.vector.memset(onesrow[:].bitcast(F32), 1.0)
            negones = wp.tile([1, N], F32R)
            nc.vector.memset(negones[:].bitcast(F32), -1.0)

            # pooled & relu'd features for the head: [128, mt(8), graph(4)]
            poolr = pp.tile([128, 8, NG], F32R)

            # ============ per-graph pipeline (software-pipelined emission) ============
            ST = {}

            def stageA(g):
                # ---- S0: load pos, round to f32r ----
                posT0 = work.tile([3, N], F32, tag="scrA")
                nc.sync.dma_start(posT0[:], posT_d[g])
                posTr = work.tile([3, N], F32R, tag="posTr")
                nc.scalar.activation(posTr[:], posT0[:], AF.Copy)

                # ---- S1: norms ----
                sq = work.tile([3, N], F32, tag="scrA")
                nc.scalar.activation(sq[:], posTr[:].bitcast(F32), AF.Square)
                x2p = psp.tile([1, N], F32, tag="ps")
                for c in range(4):
                    nc.tensor.matmul(x2p[:, 512 * c:512 * (c + 1)], ones3[:],
                                     sq[:, 512 * c:512 * (c + 1)])
                x2s = work.tile([1, N], F32R, tag="x2s")
                nc.scalar.activation(x2s[:], x2p[:], AF.Copy)
                negx2 = work.tile([1, N], F32R, tag="negx2")
                nc.scalar.activation(negx2[:], x2s[:].bitcast(F32), AF.Copy, scale=-1.0)

                # ---- S2: augmented gram operands [5, N] ----
                rhsA = work.tile([5, N], F32R, tag="rhsA")
                nc.scalar.activation(rhsA[0:3, :], posTr[:].bitcast(F32), AF.Copy)
                nc.sync.dma_start(rhsA[3:4, :], x2s[:])
                nc.sync.dma_start(rhsA[4:5, :], onesrow[:])
                lhsA = work.tile([5, N], F32R, tag="lhsA")
                nc.scalar.activation(lhsA[0:3, :], posTr[:].bitcast(F32), AF.Copy, scale=2.0)
                nc.sync.dma_start(lhsA[3:4, :], negones[:])
                nc.sync.dma_start(lhsA[4:5, :], negx2[:])

                # ---- S3: gram1 + topk1 ----
                idxall1 = work.tile([128, NT, 8], U32, tag="idxall")
                for t in range(NT):
                    ps = psp.tile([128, N], F32, tag="ps")
                    for c in range(4):
                        nc.tensor.matmul(ps[:, 512 * c:512 * (c + 1)],
                                         lhsA[:, 128 * t:128 * (t + 1)],
                                         rhsA[:, 512 * c:512 * (c + 1)])
                    vals = work.tile([128, 8], F32, tag="vals")
                    nc.vector.max(out=vals[:], in_=ps[:])
                    nc.vector.max_index(out=idxall1[:, t, :], in_max=vals[:], in_values=ps[:])

                # ---- S4: redistribute indices -> wrapped i16 [64, 640] ----
                wrap1 = _make_wrap(nc, tc, work, psp, ident, idxall1,
                                   ngroups=4, tag=f"w1_{g % 2}")
                ST[("wrap1", g)] = wrap1
                ST[("posTr", g)] = posTr

            def stageB(g):
                wrap1 = ST[("wrap1", g)]
                posTr = ST[("posTr", g)]
                # ---- S5: B1 (natural) and A1 (sigma) node features ----
                B1T = work.tile([64, N], F32, tag="BT")
                psb = psp.tile([64, N], F32, tag="ps")
                for c in range(4):
                    nc.tensor.matmul(psb[:, 512 * c:512 * (c + 1)], w1aB[:],
                                     posTr[:, 512 * c:512 * (c + 1)])
                nc.scalar.activation(B1T[:], psb[:], AF.Copy)
                A1s = work.tile([64, N], F32, tag="As")
                psa = psp.tile([64, N], F32, tag="ps")
                sig_pos = _sigma_read(posTr[:])
                for c in range(4):
                    nc.tensor.matmul(psa[:, 512 * c:512 * (c + 1)], w1aA[:],
                                     sig_pos[:, 32 * c:32 * (c + 1), :])
                nc.scalar.activation(A1s[:], psa[:], AF.Copy)

                # ---- S6+S7: conv1 MLP over 5 neighbor slabs ----
                macc = work.tile([64, N], F32, tag="macc")
                for k in range(K):
                    g1 = workB.tile([64, N], F32, tag="gslab")
                    nc.gpsimd.ap_gather(
                        out_ap=g1[:].unsqueeze(-1), in_ap=B1T[:].unsqueeze(-1),
                        idxs_ap=wrap1[:, 128 * k:128 * (k + 1)],
                        channels=64, num_elems=N, d=1, num_idxs=N)
                    nc.vector.tensor_tensor(out=g1[:], in0=g1[:], in1=A1s[:], op=ALU.add)
                    r1a = work.tile([64, N], F32R, tag="r1aslab")
                    nc.scalar.activation(r1a[:], g1[:], AF.Relu, bias=b1a[:])
                    ps1b = psp.tile([64, N], F32, tag="ps")
                    for c in range(4):
                        nc.tensor.matmul(ps1b[:, 512 * c:512 * (c + 1)], w1b[:],
                                         r1a[:, 512 * c:512 * (c + 1)])
                    r1b = work.tile([64, N], F32R, tag="r1bslab")
                    nc.scalar.activation(r1b[:], ps1b[:], AF.Relu, bias=b1b[:])
                    ps1c = psp.tile([64, N], F32, tag="ps")
                    for c in range(4):
                        nc.tensor.matmul(ps1c[:, 512 * c:512 * (c + 1)], w1c[:],
                                         r1b[:, 512 * c:512 * (c + 1)])
                    if k == 0:
                        nc.scalar.activation(macc[:], ps1c[:], AF.Copy)
                    else:
                        nc.vector.tensor_tensor(out=macc[:], in0=macc[:], in1=ps1c[:], op=ALU.max)

                # ---- x1 = bn(relu(macc + b1c)) written natural-order ----
                t1 = work.tile([64, N], F32, tag="scrA")
                nc.scalar.activation(t1[:], macc[:], AF.Relu, bias=b1c[:])
                x1nat = work.tile([64, N], F32R, tag=f"x1nat{g % 2}")
                nc.vector.tensor_scalar(
                    out=_sigma_read(x1nat[:]),
                    in0=t1[:].rearrange("c (q b) -> c q b", q=128, b=16),
                    scalar1=s1c[:], scalar2=h1c[:],
                    op0=ALU.mult, op1=ALU.add)

                ST[("x1nat", g)] = x1nat

            def stageC(g):
                x1nat = ST[("x1nat", g)]
                # ---- S8: conv2 norms (natural) ----
                sq1 = work.tile([64, N], F32, tag="scrA")
                nc.scalar.activation(sq1[:], x1nat[:].bitcast(F32), AF.Square)
                x2p2 = psp.tile([1, N], F32, tag="ps")
                for c in range(4):
                    nc.tensor.matmul(x2p2[:, 512 * c:512 * (c + 1)], ones64[:],
                                     sq1[:, 512 * c:512 * (c + 1)])
                x2c = work.tile([1, N], F32R, tag="x2s")
                nc.scalar.activation(x2c[:], x2p2[:], AF.Copy)
                negx2c = work.tile([1, N], F32R, tag="negx2")
                nc.scalar.activation(negx2c[:], x2c[:].bitcast(F32), AF.Copy, scale=-1.0)

                # ---- S9: aug operands [66, N] ----
                rhsA2 = work.tile([66, N], F32R, tag="rhsA")
                nc.scalar.activation(rhsA2[0:64, :], x1nat[:].bitcast(F32), AF.Copy)
                nc.sync.dma_start(rhsA2[64:65, :], x2c[:])
                nc.sync.dma_start(rhsA2[65:66, :], onesrow[:])
                lhsA2 = work.tile([66, N], F32R, tag="lhsA")
                nc.scalar.activation(lhsA2[0:64, :], x1nat[:].bitcast(F32), AF.Copy, scale=2.0)
                nc.sync.dma_start(lhsA2[64:65, :], negones[:])
                nc.sync.dma_start(lhsA2[65:66, :], negx2c[:])

                # ---- S10: gram2 + topk2 ----
                idxall2 = work.tile([128, NT, 8], U32, tag="idxall")
                for t in range(NT):
                    ps = psp.tile([128, N], F32, tag="ps")
                    for c in range(4):
                        nc.tensor.matmul(ps[:, 512 * c:512 * (c + 1)],
                                         lhsA2[:, 128 * t:128 * (t + 1)],
                                         rhsA2[:, 512 * c:512 * (c + 1)])
                    vals2 = work.tile([128, 8], F32, tag="vals")
                    nc.vector.max(out=vals2[:], in_=ps[:])
                    nc.vector.max_index(out=idxall2[:, t, :], in_max=vals2[:], in_values=ps[:])

                # ---- S11: redistribute ----
                wrap2 = _make_wrap(nc, tc, work, psp, ident, idxall2,
                                   ngroups=8, tag=f"w2_{g % 2}")
                ST[("wrap2", g)] = wrap2

            def stageD(g):
                wrap2 = ST[("wrap2", g)]
                x1nat = ST[("x1nat", g)]
                sig_x1 = _sigma_read(x1nat[:])
                # ---- S12: B2 (natural), A2 (sigma) ----
                B2T = work.tile([128, N], F32, tag="B2T")
                psb2 = psp.tile([128, N], F32, tag="ps")
                for c in range(4):
                    nc.tensor.matmul(psb2[:, 512 * c:512 * (c + 1)], w2B[:],
                                     x1nat[:, 512 * c:512 * (c + 1)])
                nc.scalar.activation(B2T[:], psb2[:], AF.Copy)
                A2s = work.tile([128, N], F32, tag="A2s")
                psa2 = psp.tile([128, N], F32, tag="ps")
                sig_x1 = _sigma_read(x1nat[:])
                for c in range(4):
                    nc.tensor.matmul(psa2[:, 512 * c:512 * (c + 1)], w2A[:],
                                     sig_x1[:, 32 * c:32 * (c + 1), :])
                nc.scalar.activation(A2s[:], psa2[:], AF.Copy)

                # ---- S13+S14: gather-max + combine ----
                macc2 = work.tile([128, N], F32, tag="macc")
                for k in range(K):
                    g2 = workB.tile([128, N], F32, tag="gslab")
                    nc.gpsimd.ap_gather(
                        out_ap=g2[:].unsqueeze(-1), in_ap=B2T[:].unsqueeze(-1),
                        idxs_ap=wrap2[:, 128 * k:128 * (k + 1)],
                        channels=128, num_elems=N, d=1, num_idxs=N)
                    if k == 0:
                        nc.scalar.activation(macc2[:], g2[:], AF.Copy)
                    else:
                        nc.vector.tensor_tensor(out=macc2[:], in0=macc2[:], in1=g2[:], op=ALU.max)
                nc.vector.tensor_tensor(out=macc2[:], in0=macc2[:], in1=A2s[:], op=ALU.add)
                x2sg = work.tile([128, N], F32R, tag="x2sg")
                nc.scalar.activation(x2sg[:], macc2[:], AF.Relu, bias=b2[:])

                ST[("x2sg", g)] = x2sg

            def stageE(g):
                x2sg = ST[("x2sg", g)]
                x1nat = ST[("x1nat", g)]
                sig_x1 = _sigma_read(x1nat[:])
                # ---- S15: linear-l + global max pool ----
                for mt in range(8):
                    psl = psp.tile([128, N], F32, tag="ps")
                    for c in range(4):
                        nc.tensor.matmul(psl[:, 512 * c:512 * (c + 1)],
                                         wl1[:, 128 * mt:128 * (mt + 1)],
                                         sig_x1[:, 32 * c:32 * (c + 1), :],
                                         start=True, stop=False)
                    for c in range(4):
                        nc.tensor.matmul(psl[:, 512 * c:512 * (c + 1)],
                                         wl2[:, 128 * mt:128 * (mt + 1)],
                                         x2sg[:, 512 * c:512 * (c + 1)],
                                         start=False, stop=True)
                    pr = work.tile([128, 1], F32, tag="poolred")
                    nc.vector.tensor_reduce(pr[:], psl[:], axis=AX.X, op=ALU.max)
                    nc.scalar.activation(poolr[:, mt, g:g + 1], pr[:],
                                         AF.Relu, bias=bl[:, mt:mt + 1])

            stageA(0)
            stageB(0)
            stageC(0)
            for g in range(NG):
                if g + 1 < NG:
                    stageA(g + 1)
                stageD(g)
                if g + 1 < NG:
                    stageB(g + 1)
                stageE(g)
                if g + 1 < NG:
                    stageC(g + 1)

            # ============ head MLP (all graphs) ============
            rm1 = pp.tile([128, 4, NG], F32R)
            for mt in range(4):
                ph = psp.tile([128, NG], F32, tag="ps")
                for kk in range(8):
                    nc.tensor.matmul(ph[:], wm1[:, kk, 128 * mt:128 * (mt + 1)],
                                     poolr[:, kk, :], start=(kk == 0), stop=(kk == 7))
                nc.scalar.activation(rm1[:, mt, :], ph[:], AF.Relu,
                                     bias=bm1[:, mt:mt + 1])
            rm2 = pp.tile([128, 2, NG], F32R)
            for mt in range(2):
                ph = psp.tile([128, NG], F32, tag="ps")
                for kk in range(4):
                    nc.tensor.matmul(ph[:], wm2[:, kk, 128 * mt:128 * (mt + 1)],
                                     rm1[:, kk, :], start=(kk == 0), stop=(kk == 3))
                nc.scalar.activation(rm2[:, mt, :], ph[:], AF.Relu,
                                     bias=bm2[:, mt:mt + 1])
            pho = psp.tile([2, NG], F32, tag="ps")
            for kk in range(2):
                nc.tensor.matmul(pho[:], wout[:, kk, :], rm2[:, kk, :],
                                 start=(kk == 0), stop=(kk == 1))
            outs = pp.tile([2, NG], F32)
            nc.vector.tensor_scalar_add(outs[:], pho[:], bout[:])
            nc.sync.dma_start(out_d[:], outs[:])

    nc.compile()
    return nc


def _make_wrap(nc, tc, work, psp, ident, idxall, ngroups, tag):
    """[128, 16, 8] u32 find_index8 results -> wrapped i16 [16*ngroups, 640] for ap_gather.

    Edge order m = 2048*k + 16*q + b: node i = 128*b + q, slot k.
    """
    F32_ = mybir.dt.float32
    I16_ = mybir.dt.int16
    idxf = work.tile([128, 5, 16], F32_, tag=tag + "idxf")
    nc.vector.tensor_copy(idxf[:], idxall[:, :, 0:5].transpose([0, 2, 1]))
    tp = psp.tile([80, 128], F32_, tag="ps")
    nc.tensor.transpose(tp[:], idxf[:].rearrange("p a b -> p (a b)"), ident[:])
    idxt16 = work.tile([80, 128], I16_, tag=tag + "idxt16")
    nc.vector.tensor_copy(idxt16[:], tp[:])
    wrap = work.tile([16 * ngroups, 640], I16_, tag=tag + "wrap")
    for gg in range(ngroups):
        for k in range(5):
            nc.sync.dma_start(wrap[16 * gg:16 * (gg + 1), 128 * k:128 * (k + 1)],
                              idxt16[16 * k:16 * k + 16, :])
    return wrap


def _fold_weights(inp):
    """Host-side BN folding / edge-weight splitting. Layout-only + tiny weight algebra."""
    f = {k: np.asarray(v, dtype=np.float64) for k, v in inp.items()}
    w = {}
    # conv1 layer a: e @ W1a = x_i @ (Wtop - Wbot) + x_j @ Wbot
    w["w1aA"] = (f["w1a"][:3] - f["w1a"][3:])
    w["w1aB"] = f["w1a"][3:]
    w["b1a"] = f["b1a"]
    # fold (s1a, h1a) into layer b; (s1b, h1b) into layer c
    w["w1b"] = f["s1a"][:, None] * f["w1b"]
    w["b1b"] = f["h1a"] @ f["w1b"] + f["b1b"]
    w["w1c"] = f["s1b"][:, None] * f["w1c"]
    w["b1c"] = f["h1b"] @ f["w1c"] + f["b1c"]
    w["s1c"], w["h1c"] = f["s1c"], f["h1c"]
    # conv2
    w["w2A"] = f["w2"][:64] - f["w2"][64:]
    w["w2B"] = f["w2"][64:]
    w["b2"] = f["b2"]
    # linear l: x1-part plain; x2-part folded with (s2, h2)
    wl1 = f["wl"][:64]
    wl2 = f["s2"][:, None] * f["wl"][64:]
    blf = f["bl"] + f["h2"] @ f["wl"][64:]
    w["wl1"], w["wl2"], w["bl"] = wl1, wl2, blf
    # head: fold (sl, hl) into m1; (sm1, hm1) into m2; (sm2, hm2) into out
    w["wm1"] = f["sl"][:, None] * f["wm1"]
    w["bm1"] = f["hl"] @ f["wm1"] + f["bm1"]
    w["wm2"] = f["sm1"][:, None] * f["wm2"]
    w["bm2"] = f["hm1"] @ f["wm2"] + f["bm2"]
    w["wout"] = f["sm2"][:, None] * f["wout"]
    w["bout"] = f["hm2"] @ f["wout"] + f["bout"]
    return {k: v.astype(np.float32) for k, v in w.items()}


def _weight_maps(w):
    m = {}
    m["w1aA"] = w["w1aA"]
    m["w1aB"] = w["w1aB"]
    m["w1b"] = w["w1b"]
    m["w1c"] = w["w1c"]
    m["w2A"] = w["w2A"]
    m["w2B"] = w["w2B"]
    m["wl1"] = w["wl1"]
    m["wl2"] = w["wl2"]
    m["wm1"] = np.ascontiguousarray(w["wm1"].reshape(8, 128, 512).transpose(1, 0, 2))
    m["wm2"] = np.ascontiguousarray(w["wm2"].reshape(4, 128, 256).transpose(1, 0, 2))
    m["wout"] = np.ascontiguousarray(w["wout"].reshape(2, 128, 2).transpose(1, 0, 2))
    m["b1a"] = w["b1a"].reshape(64, 1)
    m["b1b"] = w["b1b"].reshape(64, 1)
    m["b1c"] = w["b1c"].reshape(64, 1)
    m["s1c"] = w["s1c"].reshape(64, 1)
    m["h1c"] = w["h1c"].reshape(64, 1)
    m["b2"] = w["b2"].reshape(128, 1)
    m["bl"] = np.ascontiguousarray(w["bl"].reshape(8, 128).T)
    m["bm1"] = np.ascontiguousarray(w["bm1"].reshape(4, 128).T)
    m["bm2"] = np.ascontiguousarray(w["bm2"].reshape(2, 128).T)
    m["bout"] = w["bout"].reshape(2, 1)
    return {k: np.ascontiguousarray(v, dtype=np.float32) for k, v in m.items()}


def kernel(**inputs):
    if "nc" not in _CACHE:
        _CACHE["nc"] = build_nc()
    nc = _CACHE["nc"]

    w = _fold_weights(inputs)
    wm = _weight_maps(w)
    pos = np.asarray(inputs["pos"], dtype=np.float32)  # [32, 2048, 3]
    B = pos.shape[0]

    in_maps = []
    for c in range(NCORES):
        m = dict(wm)
        m["posT"] = np.ascontiguousarray(pos[NG * c:NG * (c + 1)].transpose(0, 2, 1))
        in_maps.append(m)

    res = bass_utils.run_bass_kernel_spmd(nc, in_maps, core_ids=list(range(NCORES)))
    out = np.zeros((B, 2), dtype=np.float32)
    for c in range(NCORES):
        out[NG * c:NG * (c + 1)] = res.results[c]["out"].T
    return out

